# revision 1
# baseline (speedup 1.0000x reference)
"""Trainium2 Bass kernel for nn_BertEncoder_403726926494.

Reference computation (per batch element):
  - ragged sentence extraction from hidden_states, masked-softmax attention
    pooling per sentence with W_doc            -> doc_pooled [B, D, H]
  - query extraction (rows 1..32), masked-softmax pooling with W_query
    broadcast over D                           -> q_bcast   [B, D, H]

Device strategy (SPMD, one program on 8 cores, 8 batch elements per core):
  - Per core-slot, DMA only the used row-span of hidden_states into SBUF
    (slots are assigned from a global sort of spans so the per-slot span is
    a static program constant shared by all cores).
  - Per-token scores s[t] = x_t . W_doc: DVE/GpSimd tensor_tensor multiply
    against a W-broadcast tile, then a free-dim reduce on ACT (activation
    Copy + accum_out) or DVE (tensor_reduce) -- engine choice per slot to
    balance load.
  - softmax without max-subtraction (scores are O(1)):
      alphaU[t,j] = exp(s[t] + logSel[t,j])   one ACT op per chunk, where
    logSel is a host-built {0, -1e30} mask marking token t in sentence j
    (columns padded to 32 with -1e30).
      num[j,:H] | den[j] = alphaU^T @ [X | 1]  PE matmul with a ones-column
    appended to X; 4 slots share one PSUM tile via tile_position col-groups.
  - out[j] = num[j] / (den[j] + eps)  (eps keeps empty sentences at 0).
  - Query path packs 4 examples x 32 query rows onto 128 partitions; the
    query-length mask and example-block structure fold into one host-built
    log-mask. q_pooled is broadcast over D on the host.
  - b_doc / b_query shift every score in a softmax segment equally, so they
    cancel and are ignored.
"""

import numpy as np

B, L, H = 64, 512, 768
D, S, Q = 16, 64, 32
NCORES = 8
SLOTS = 8
MPAD = 32  # selector columns padded to one PE col-group
NEG_BIAS = -1.0e30
DEN_EPS = 1.0e-30

# Engine assignment knobs (tuned from traces):
#   score TT multiply per slot: "dve" or "gps"
#   score reduce per slot: "act" (per-chunk accum) or "dve" (merged reduce)
TT_ENGINE = ["dve"] * SLOTS
RED_ENGINE = ["act", "act", "act", "act", "act", "act", "dve", "dve"]
Q_RED_ENGINE = "act"

_compiled: dict = {}


def _slot_geometry(slot_spans):
    nts = [(sp + 127) // 128 for sp in slot_spans]
    rems = [sp - 128 * (nt - 1) for sp, nt in zip(slot_spans, nts)]
    coffs = [0]
    for nt in nts:
        coffs.append(coffs[-1] + nt)
    return nts, rems, coffs


def _build(slot_spans):
    """Build + compile the SPMD Bass program for the given per-slot spans."""
    from contextlib import ExitStack

    import concourse.bacc as bacc
    import concourse.tile as tile
    from concourse import mybir

    f32 = mybir.dt.float32
    MULT = mybir.AluOpType.mult
    ADD = mybir.AluOpType.add
    EXP = mybir.ActivationFunctionType.Exp
    COPY = mybir.ActivationFunctionType.Copy

    nts, rems, coffs = _slot_geometry(slot_spans)
    ntsum = coffs[-1]
    foffs = [0]
    for nt in nts:
        foffs.append(foffs[-1] + nt - 1)
    roffs = [0]
    for r in rems:
        roffs.append(roffs[-1] + r)

    nc = bacc.Bacc(
        "TRN2", target_bir_lowering=False, debug=False, num_devices=NCORES
    )
    nfull = sum(nt - 1 for nt in nts)
    nremtot = sum(rems)
    sfull = nc.dram_tensor(
        "sfull", [128, max(nfull, 1), H], f32, kind="ExternalInput"
    ).ap()
    srem = nc.dram_tensor("srem", [nremtot, H], f32, kind="ExternalInput").ap()
    qstage = nc.dram_tensor("qstage", [2, 128, H], f32, kind="ExternalInput").ap()
    wd = nc.dram_tensor("wd", [1, H], f32, kind="ExternalInput").ap()
    wq = nc.dram_tensor("wq", [1, H], f32, kind="ExternalInput").ap()
    selt = nc.dram_tensor(
        "selt", [128, ntsum, MPAD], f32, kind="ExternalInput"
    ).ap()
    qmask = nc.dram_tensor("qmask", [128, 2, MPAD], f32, kind="ExternalInput").ap()
    doc_out = nc.dram_tensor("doc_out", [SLOTS, D, H], f32, kind="ExternalOutput").ap()
    q_out = nc.dram_tensor("q_out", [SLOTS, H], f32, kind="ExternalOutput").ap()

    with tile.TileContext(nc) as tc, ExitStack() as ctx:
        const = ctx.enter_context(tc.tile_pool(name="const", bufs=1))

        wrow_d = const.tile([1, H], f32)
        nc.sync.dma_start(out=wrow_d[:], in_=wd[:])
        wrow_q = const.tile([1, H], f32)
        nc.sync.dma_start(out=wrow_q[:], in_=wq[:])
        selt_t = const.tile([128, ntsum, MPAD], f32)
        nc.sync.dma_start(out=selt_t[:], in_=selt[:])
        qmask_t = const.tile([128, 2, MPAD], f32)
        nc.sync.dma_start(out=qmask_t[:], in_=qmask[:])

        # Broadcast W rows across all 128 partitions (gpsimd custom op).
        wb_d = const.tile([128, H], f32)
        wb_q = const.tile([128, H], f32)
        nc.gpsimd.partition_broadcast(wb_d[:], wrow_d[:])
        nc.gpsimd.partition_broadcast(wb_q[:], wrow_q[:])

        xpool = ctx.enter_context(tc.tile_pool(name="xp", bufs=8))
        apool = ctx.enter_context(tc.tile_pool(name="apl", bufs=4))
        scrp = ctx.enter_context(tc.tile_pool(name="scr", bufs=2))
        outp = ctx.enter_context(tc.tile_pool(name="outp", bufs=2))
        smallp = ctx.enter_context(tc.tile_pool(name="smallp", bufs=4))
        qpoolp = ctx.enter_context(tc.tile_pool(name="qpl", bufs=2))
        nump = ctx.enter_context(tc.tile_pool(name="nump", bufs=2, space="PSUM"))
        qnump = ctx.enter_context(tc.tile_pool(name="qnump", bufs=1, space="PSUM"))

        # ---- scores: xw = x * W_bcast (TT), then free-dim reduce -> scol ----
        def emit_scores(x_ap_full, nt, rem, scol, wb, name, tt_eng, red_eng):
            # x_ap_full: [128, nt, H(+1)] view; uses cols 0:H
            xw = scrp.tile([128, nt, H], f32, tag="scratch", name=f"xw{name}")
            tt = nc.gpsimd if tt_eng == "gps" else nc.vector
            if nt > 1:
                tt.tensor_tensor(
                    out=xw[:, 0 : nt - 1, :],
                    in0=x_ap_full[:, 0 : nt - 1, 0:H],
                    in1=wb[:].rearrange("p (o h) -> p o h", o=1).broadcast_to(
                        [128, nt - 1, H]
                    ),
                    op=MULT,
                )
            tt.tensor_tensor(
                out=xw[0:rem, nt - 1, :],
                in0=x_ap_full[0:rem, nt - 1, 0:H],
                in1=wb[0:rem, :],
                op=MULT,
            )
            if red_eng == "dve":
                if nt > 1:
                    nc.vector.tensor_reduce(
                        out=scol[:, 0 : nt - 1],
                        in_=xw[:, 0 : nt - 1, :],
                        axis=mybir.AxisListType.X,
                        op=ADD,
                    )
                nc.vector.tensor_reduce(
                    out=scol[0:rem, nt - 1 : nt],
                    in_=xw[0:rem, nt - 1, :],
                    axis=mybir.AxisListType.X,
                    op=ADD,
                )
            else:
                s2 = scrp.tile([128, H], f32, tag="scratch2", name=f"s2{name}")
                for c in range(nt):
                    cnt = 128 if c < nt - 1 else rem
                    nc.scalar.activation(
                        s2[0:cnt, :], xw[0:cnt, c, :], COPY,
                        bias=0.0, scale=1.0,
                        accum_out=scol[0:cnt, c : c + 1],
                    )

        # ---- doc slots: per-slot pipeline; two groups of 4 share PSUM tiles
        # via PE col-groups. Slots are emitted alternating between the two
        # groups so independent work overlaps and consecutive slots' matmuls
        # land on different col-groups (concurrent PE streams).
        numgs = {}

        xtiles = {}

        def load_slot(s):
            nt, rem = nts[s], rems[s]
            x = xpool.tile([128, nt, H + 1], f32, tag="x", name=f"x{s}")
            if nt > 1:
                nc.sync.dma_start(
                    out=x[:, 0 : nt - 1, 0:H],
                    in_=sfull[:, foffs[s] : foffs[s] + nt - 1, :],
                )
            nc.sync.dma_start(
                out=x[0:rem, nt - 1, 0:H],
                in_=srem[roffs[s] : roffs[s] + rem, :],
            )
            nc.vector.memset(x[:, :, H : H + 1], 1.0)
            xtiles[s] = x

        def emit_slot(s):
            g, k = divmod(s, 4)
            if g not in numgs:
                numgs[g] = nump.tile([128, 1024], f32, tag="num", name=f"num{g}")
            numg = numgs[g]
            nt, rem, coff = nts[s], rems[s], coffs[s]
            x = xtiles[s]

            scol = smallp.tile([128, nt], f32, tag="scol", name=f"scol{s}")
            emit_scores(
                x[:], nt, rem, scol, wb_d, f"d{s}", TT_ENGINE[s], RED_ENGINE[s]
            )

            at = apool.tile([128, nt, MPAD], f32, tag="at", name=f"at{s}")
            for c in range(nt):
                cnt = 128 if c < nt - 1 else rem
                nc.scalar.activation(
                    at[0:cnt, c, :],
                    selt_t[0:cnt, coff + c, :],
                    EXP,
                    bias=scol[0:cnt, c : c + 1],
                    scale=1.0,
                )
            for c in range(nt):
                cnt = 128 if c < nt - 1 else rem
                first, last = c == 0, c == nt - 1
                nc.tensor.matmul(
                    numg[32 * k : 32 * k + MPAD, 0:512],
                    at[0:cnt, c, :],
                    x[0:cnt, c, 0:512],
                    start=first, stop=last,
                    tile_position=(0, 32 * k),
                    skip_group_check=True,
                )
                nc.tensor.matmul(
                    numg[32 * k : 32 * k + MPAD, 512 : H + 1],
                    at[0:cnt, c, :],
                    x[0:cnt, c, 512 : H + 1],
                    start=first, stop=last,
                    tile_position=(0, 32 * k),
                    skip_group_check=True,
                )

        def finish_group(g):
            numg = numgs[g]
            de = smallp.tile([128, 1], f32, tag="de", name=f"de{g}")
            nc.vector.tensor_scalar(
                out=de[:], in0=numg[:, H : H + 1], scalar1=DEN_EPS,
                scalar2=None, op0=ADD,
            )
            rec = smallp.tile([128, 1], f32, tag="rec", name=f"rec{g}")
            nc.vector.reciprocal(rec[:], de[:])
            do = outp.tile([128, H], f32, tag="do", name=f"do{g}")
            nc.scalar.activation(
                do[:], numg[:, 0:H], COPY, bias=0.0, scale=rec[:, 0:1]
            )
            for k in range(4):
                nc.scalar.dma_start(
                    out=doc_out[4 * g + k, :, :],
                    in_=do[32 * k : 32 * k + D, :],
                )

        # ---- query: two batches of 4 examples x 32 rows -> one PSUM tile ----
        def emit_query(qnumg, b):
            qpack = qpoolp.tile([128, H + 1], f32, tag="qpack", name=f"qpack{b}")
            nc.sync.dma_start(out=qpack[:, 0:H], in_=qstage[b, :, :])
            nc.vector.memset(qpack[:, H : H + 1], 1.0)
            qscol = smallp.tile([128, 1], f32, tag="qscol", name=f"qscol{b}")
            emit_scores(
                qpack[:].rearrange("p (o h) -> p o h", o=1), 1, 128, qscol, wb_q,
                f"q{b}", "dve", Q_RED_ENGINE,
            )
            qat = apool.tile([128, MPAD], f32, tag="qat", name=f"qat{b}")
            nc.scalar.activation(
                qat[:], qmask_t[:, b, :], EXP, bias=qscol[:, 0:1], scale=1.0
            )
            nc.tensor.matmul(
                qnumg[32 * b : 32 * b + MPAD, 0:512],
                qat[:], qpack[:, 0:512],
                start=True, stop=True, tile_position=(0, 32 * b),
            )
            nc.tensor.matmul(
                qnumg[32 * b : 32 * b + MPAD, 512 : H + 1],
                qat[:], qpack[:, 512 : H + 1],
                start=True, stop=True, tile_position=(0, 32 * b),
            )

        qnumg = qnump.tile([64, 1024], f32, tag="qnum", name="qnum")
        for s in range(SLOTS):
            load_slot(s)
        for s in (0, 4, 1, 5):
            emit_slot(s)
        emit_query(qnumg, 0)
        for s in (2, 6, 3, 7):
            emit_slot(s)
        emit_query(qnumg, 1)
        finish_group(0)
        finish_group(1)

        qde = smallp.tile([64, 1], f32, tag="qde", name="qde")
        nc.vector.tensor_scalar(
            out=qde[:], in0=qnumg[:, H : H + 1], scalar1=DEN_EPS,
            scalar2=None, op0=ADD,
        )
        qrec = smallp.tile([64, 1], f32, tag="qrec", name="qrec")
        nc.vector.reciprocal(qrec[:], qde[:])
        qo = outp.tile([64, H], f32, tag="qo", name="qo")
        nc.scalar.activation(
            qo[:], qnumg[:, 0:H], COPY, bias=0.0, scale=qrec[:, 0:1]
        )
        for b in range(2):
            nc.sync.dma_start(
                out=q_out[4 * b : 4 * b + 4, :],
                in_=qo[32 * b : 32 * b + 4, :],
            )

    nc.compile()
    return nc


def _prepare(query_len, seq_lens):
    """Host-side geometry: spans, slot assignment, selector/mask arrays."""
    ql = np.asarray(query_len).astype(np.int64)
    sl = np.asarray(seq_lens).astype(np.int64)
    offs = ql[:, None] + 2 + np.cumsum(sl, axis=1) - sl  # [B, D] sentence starts
    end = ql + 2 + sl.sum(axis=1)
    span = np.maximum(end, 1 + Q)  # query rows 1..32 must be covered
    order = np.argsort(-span, kind="stable")  # rank -> example id
    slot_spans = tuple(int(span[order[8 * s]]) for s in range(SLOTS))
    nts, rems, coffs = _slot_geometry(slot_spans)
    ntsum = coffs[-1]

    selt_all = np.full((NCORES, 128, ntsum, MPAD), NEG_BIAS, np.float32)
    qmask_all = np.full((NCORES, 128, 2, MPAD), NEG_BIAS, np.float32)
    ex_map = np.empty((NCORES, SLOTS), np.int64)
    for c in range(NCORES):
        for s in range(SLOTS):
            e = int(order[8 * s + c])
            ex_map[c, s] = e
            for j in range(D):
                ln = int(sl[e, j])
                if ln == 0:
                    continue
                o = int(offs[e, j])
                t = np.arange(o, o + ln)
                selt_all[c, t % 128, coffs[s] + t // 128, j] = 0.0
            b, sub = divmod(s, 4)
            qmask_all[c, 32 * sub : 32 * sub + int(ql[e]), b, sub] = 0.0
    return slot_spans, ex_map, selt_all, qmask_all


def kernel(hidden_states, W_doc, b_doc, W_query, b_query, query_len, seq_lens):
    hs = np.ascontiguousarray(np.asarray(hidden_states, dtype=np.float32))
    wd = np.ascontiguousarray(np.asarray(W_doc, np.float32).reshape(1, H))
    wq = np.ascontiguousarray(np.asarray(W_query, np.float32).reshape(1, H))

    slot_spans, ex_map, selt_all, qmask_all = _prepare(query_len, seq_lens)

    nc = _compiled.get(slot_spans)
    if nc is None:
        nc = _build(slot_spans)
        _compiled[slot_spans] = nc

    nts, rems, _ = _slot_geometry(slot_spans)
    nfull = sum(nt - 1 for nt in nts)
    nremtot = sum(rems)

    in_maps = []
    for c in range(NCORES):
        sfull = np.empty((128, max(nfull, 1), H), np.float32)
        srem = np.empty((nremtot, H), np.float32)
        qstage = np.empty((2, 128, H), np.float32)
        fo = ro = 0
        for s in range(SLOTS):
            e = int(ex_map[c, s])
            nt, rem = nts[s], rems[s]
            if nt > 1:
                sfull[:, fo : fo + nt - 1, :] = (
                    hs[e, 0 : (nt - 1) * 128, :]
                    .reshape(nt - 1, 128, H)
                    .transpose(1, 0, 2)
                )
                fo += nt - 1
            srem[ro : ro + rem] = hs[e, (nt - 1) * 128 : (nt - 1) * 128 + rem, :]
            ro += rem
            b, sub = divmod(s, 4)
            qstage[b, 32 * sub : 32 * sub + 32, :] = hs[e, 1 : 1 + Q, :]
        in_maps.append(
            {
                "sfull": sfull,
                "srem": srem,
                "qstage": qstage,
                "wd": wd,
                "wq": wq,
                "selt": selt_all[c],
                "qmask": qmask_all[c],
            }
        )

    from concourse.bass_utils import run_bass_kernel_spmd

    res = run_bass_kernel_spmd(nc, in_maps, list(range(NCORES)))

    doc = np.empty((B, D, H), np.float32)
    qp = np.empty((B, H), np.float32)
    for c in range(NCORES):
        r = res.results[c]
        for s in range(SLOTS):
            e = int(ex_map[c, s])
            doc[e] = r["doc_out"][s]
            qp[e] = r["q_out"][s]
    q_bcast = np.broadcast_to(qp[:, None, :], (B, D, H))
    return doc, q_bcast



# revision 7
# speedup vs baseline: 1.3308x; 1.3308x over previous
"""Trainium2 Bass kernel for nn_BertEncoder_403726926494.

Reference computation (per batch element):
  - ragged sentence extraction from hidden_states, masked-softmax attention
    pooling per sentence with W_doc            -> doc_pooled [B, D, H]
  - query extraction (rows 1..32), masked-softmax pooling with W_query
    broadcast over D                           -> q_bcast   [B, D, H]

Device strategy (SPMD, one program on 8 cores, 8 batch elements per core):
  - Dense token packing: each core receives ONE fp16 token stream = the
    concatenation of its 8 examples' doc-sentence tokens, padded to a 128
    boundary, followed by a copy of their query tokens.  Token t lands on
    SBUF partition t%128 of chunk t//128.  A trailing ones-column (col 768)
    rides along for the softmax denominators.
  - Per chunk: scores s_t = x_t . W via a tensor_tensor multiply plus a
    free-dim reduce (engines chosen per chunk to balance DVE/ACT/GpSimd),
    exp over score groups, then at[t,m] = exp(s_t) * onehot[t,m] with a
    host-built one-hot selector (m = pooling column of token t).
  - Doc pooling columns: 8 examples x 16 sentences = exactly 128 PE columns,
    so ONE PSUM accumulation group [128, 769] collects num|den for every
    sentence of the core across all doc chunks (fp16 matmuls, fp32 PSUM).
    Query chunks accumulate into a second small group [32, 769] (8 cols used).
  - out = num * (1/(den+eps)); results leave as fp16 and are scattered back
    to [B, D, H] on the host.  b_doc / b_query shift every score in a softmax
    segment equally, so they cancel and are ignored.
"""

import numpy as np

B, L, H = 64, 512, 768
D, S, Q = 16, 64, 32
NCORES = 8
EPB = 8  # examples per core
DEN_EPS = 1.0e-30

F16 = np.float16

# per-chunk engine knobs, tuned from traces; index = chunk id (doc then q)
TT_GPS_CHUNKS = {3, 7}       # tensor_tensor multiply on GpSimd for these
RED_ACT_CHUNKS = {0, 4, 8, 11}  # score reduce on ACT (others DVE)
EXP_GROUPS = ((0, 4), (4, 8), (8, 11), (11, 13))

_compiled: dict = {}


def _exp_groups(NT):
    gs = [(a, min(b, NT)) for a, b in EXP_GROUPS if a < NT]
    if gs and gs[-1][1] < NT:
        gs.append((gs[-1][1], NT))
    if not gs:
        gs = [(0, NT)]
    return gs


def _build(NTD, NTQ, REM_D, REM_Q):
    """Build + compile the SPMD Bass program for the given chunk geometry."""
    from contextlib import ExitStack

    import concourse.bacc as bacc
    import concourse.tile as tile
    from concourse import mybir

    f32 = mybir.dt.float32
    f16 = mybir.dt.float16
    MULT = mybir.AluOpType.mult
    ADD = mybir.AluOpType.add
    EXP = mybir.ActivationFunctionType.Exp
    COPY = mybir.ActivationFunctionType.Copy
    AXF = mybir.AxisListType.X

    NT = NTD + NTQ
    W = H + 1  # 769: H data cols + ones col

    nc = bacc.Bacc(
        "TRN2", target_bir_lowering=False, debug=False, num_devices=NCORES
    )
    xdf = nc.dram_tensor("xdf", [128, max(NTD - 1, 1), W], f16, kind="ExternalInput").ap()
    xdr = nc.dram_tensor("xdr", [REM_D, W], f16, kind="ExternalInput").ap()
    xqf = nc.dram_tensor("xqf", [128, max(NTQ - 1, 1), W], f16, kind="ExternalInput").ap()
    xqr = nc.dram_tensor("xqr", [REM_Q, W], f16, kind="ExternalInput").ap()
    onehot = nc.dram_tensor("onehot", [128, NT, 128], f16, kind="ExternalInput").ap()
    wbd = nc.dram_tensor("wbd", [128, H], f16, kind="ExternalInput").ap()
    wbq = nc.dram_tensor("wbq", [128, H], f16, kind="ExternalInput").ap()
    doc_out = nc.dram_tensor("doc_out", [128, H], f16, kind="ExternalOutput").ap()
    q_out = nc.dram_tensor("q_out", [EPB, H], f16, kind="ExternalOutput").ap()

    with tile.TileContext(nc) as tc, ExitStack() as ctx:
        const = ctx.enter_context(tc.tile_pool(name="const", bufs=1))
        atp = ctx.enter_context(tc.tile_pool(name="atp", bufs=3))
        scrp = ctx.enter_context(tc.tile_pool(name="scr", bufs=2))
        outp = ctx.enter_context(tc.tile_pool(name="outp", bufs=2))
        smallp = ctx.enter_context(tc.tile_pool(name="smallp", bufs=4))
        nump = ctx.enter_context(tc.tile_pool(name="nump", bufs=1, space="PSUM"))
        qnump = ctx.enter_context(tc.tile_pool(name="qnump", bufs=1, space="PSUM"))

        x = const.tile([128, NT, W], f16)
        oh = const.tile([128, NT, 128], f16)
        wb_d = const.tile([128, H], f16)
        wb_q = const.tile([128, H], f16)
        scol = const.tile([128, NT], f32)
        ecol = const.tile([128, NT], f32)
        s2 = const.tile([128, H], f16)  # dummy out for ACT accum reduce

        # zero the partial chunks up front so padding rows stay finite
        # (zero onehot rows keep them out of the pools; engine partition
        # access must start at 0, so clear whole chunks, then partial-DMA)
        if REM_D < 128:
            nc.vector.memset(x[:, NTD - 1, :], 0.0)
        if REM_Q < 128:
            nc.vector.memset(x[:, NT - 1, :], 0.0)

        # ---- input DMAs (sync ring; issue order == stream order) ----
        nc.sync.dma_start(out=wb_d[:], in_=wbd[:])
        nc.sync.dma_start(out=oh[:], in_=onehot[:])
        nfd = NTD - 1
        bounds = sorted({min(g, nfd) for g in (4, 7, nfd)} | {0})
        for a, b in zip(bounds[:-1], bounds[1:]):
            if b > a:
                nc.sync.dma_start(out=x[:, a:b, :], in_=xdf[:, a:b, :])
        nc.sync.dma_start(out=x[0:REM_D, NTD - 1, :], in_=xdr[:])
        nc.sync.dma_start(out=wb_q[:], in_=wbq[:])
        if NTQ > 1:
            nc.sync.dma_start(out=x[:, NTD : NT - 1, :], in_=xqf[:])
        nc.sync.dma_start(out=x[0:REM_Q, NT - 1, :], in_=xqr[:])

        numg = nump.tile([128, 1024], f32, tag="num", name="num")
        qnumg = qnump.tile([32, 1024], f32, tag="qnum", name="qnum")

        def emit_scores(c):
            wb = wb_d if c < NTD else wb_q
            xw = scrp.tile([128, H], f16, tag="xw", name=f"xw{c}")
            tt = nc.gpsimd if c in TT_GPS_CHUNKS else nc.vector
            tt.tensor_tensor(out=xw[:], in0=x[:, c, 0:H], in1=wb[:], op=MULT)
            if c in RED_ACT_CHUNKS:
                nc.scalar.activation(
                    s2[:], xw[:], COPY, bias=0.0, scale=1.0,
                    accum_out=scol[:, c : c + 1],
                )
            else:
                nc.vector.tensor_reduce(
                    out=scol[:, c : c + 1], in_=xw[:], axis=AXF, op=ADD
                )

        def emit_pool(c):
            mcols = 128 if c < NTD else 32
            psum = numg if c < NTD else qnumg
            start = c == 0 or c == NTD
            stop = c == NTD - 1 or c == NT - 1
            at = atp.tile([128, 128], f16, tag="at", name=f"at{c}")
            nc.gpsimd.tensor_scalar_mul(
                at[:, 0:mcols], oh[:, c, 0:mcols], ecol[:, c : c + 1]
            )
            nc.tensor.matmul(
                psum[0:mcols, 0:512], at[:, 0:mcols], x[:, c, 0:512],
                start=start, stop=stop,
            )
            nc.tensor.matmul(
                psum[0:mcols, 512:W], at[:, 0:mcols], x[:, c, 512:W],
                start=start, stop=stop,
            )

        for a, b in _exp_groups(NT):
            for c in range(a, b):
                emit_scores(c)
            nc.scalar.activation(ecol[:, a:b], scol[:, a:b], EXP, bias=0.0, scale=1.0)
            for c in range(a, b):
                emit_pool(c)

        # ---- doc finish: out = num / (den + eps) ----
        de = smallp.tile([128, 1], f32, tag="de", name="de")
        nc.vector.tensor_scalar(
            out=de[:], in0=numg[:, H : H + 1], scalar1=DEN_EPS, scalar2=None, op0=ADD
        )
        rec = smallp.tile([128, 1], f32, tag="rec", name="rec")
        nc.vector.reciprocal(rec[:], de[:])
        do = outp.tile([128, H], f16, tag="do", name="do")
        nc.scalar.activation(do[:], numg[:, 0:H], COPY, bias=0.0, scale=rec[:, 0:1])
        nc.scalar.dma_start(out=doc_out[:], in_=do[:])

        # ---- query finish ----
        qde = smallp.tile([EPB, 1], f32, tag="qde", name="qde")
        nc.vector.tensor_scalar(
            out=qde[:], in0=qnumg[0:EPB, H : H + 1], scalar1=DEN_EPS,
            scalar2=None, op0=ADD,
        )
        qrec = smallp.tile([EPB, 1], f32, tag="qrec", name="qrec")
        nc.vector.reciprocal(qrec[:], qde[:])
        qo = outp.tile([EPB, H], f16, tag="qo", name="qo")
        nc.scalar.activation(
            qo[:], qnumg[0:EPB, 0:H], COPY, bias=0.0, scale=qrec[:, 0:1]
        )
        nc.scalar.dma_start(out=q_out[:], in_=qo[:])

    nc.compile()
    return nc


def _prepare(query_len, seq_lens):
    """Host-side geometry: example->core assignment + per-core streams."""
    ql = np.asarray(query_len).astype(np.int64)
    sl = np.asarray(seq_lens).astype(np.int64)
    offs = ql[:, None] + 2 + np.cumsum(sl, axis=1) - sl  # [B, D] sentence starts
    doc_tok = sl.sum(axis=1)

    # greedy balance of total tokens into NCORES bins of EPB examples
    tot = doc_tok + ql
    order = np.argsort(-tot, kind="stable")
    ex_map = [[] for _ in range(NCORES)]
    loads = np.zeros(NCORES, np.int64)
    for e in order:
        cand = [c for c in range(NCORES) if len(ex_map[c]) < EPB]
        c = min(cand, key=lambda cc: (loads[cc], cc))
        ex_map[c].append(int(e))
        loads[c] += tot[e]

    drows = np.array([sum(doc_tok[e] for e in ex_map[c]) for c in range(NCORES)])
    qrows = np.array([sum(ql[e] for e in ex_map[c]) for c in range(NCORES)])
    NTD = int(max(-(-r // 128) for r in drows))
    NTQ = int(max(-(-r // 128) for r in qrows))
    REM_D = max(int(max(r - 128 * (NTD - 1) for r in drows)), 1)
    REM_Q = max(int(max(r - 128 * (NTQ - 1) for r in qrows)), 1)

    # per-core gather indices (into hs.reshape(B*L, H)) and pooling col ids
    plan = []
    for c in range(NCORES):
        didx, dsid, qidx, qsid = [], [], [], []
        for k, e in enumerate(ex_map[c]):
            for j in range(D):
                n = int(sl[e, j])
                if n == 0:
                    continue
                o = int(offs[e, j])
                didx.append(np.arange(e * L + o, e * L + o + n))
                dsid.append(np.full(n, 16 * k + j))
            n = int(ql[e])
            qidx.append(np.arange(e * L + 1, e * L + 1 + n))
            qsid.append(np.full(n, k))
        plan.append(
            (
                np.concatenate(didx),
                np.concatenate(dsid),
                np.concatenate(qidx),
                np.concatenate(qsid),
            )
        )
    return ex_map, plan, (NTD, NTQ, REM_D, REM_Q)


def _stage_core(hs2, plan_c, geom):
    NTD, NTQ, REM_D, REM_Q = geom
    NT = NTD + NTQ
    W = H + 1
    didx, dsid, qidx, qsid = plan_c
    nd_pad = 128 * (NTD - 1) + REM_D
    nq_pad = 128 * (NTQ - 1) + REM_Q

    def pack(idx, npad):
        xs = np.zeros((npad, W), F16)
        xs[: len(idx), 0:H] = hs2[idx]
        xs[: len(idx), H] = 1.0
        return xs

    xd = pack(didx, nd_pad)
    xq = pack(qidx, nq_pad)
    xdf = np.ascontiguousarray(
        xd[: 128 * (NTD - 1)].reshape(max(NTD - 1, 1), -1, W)[:, :128].transpose(1, 0, 2)
        if NTD > 1
        else np.zeros((128, 1, W), F16)
    )
    xdr = np.ascontiguousarray(xd[128 * (NTD - 1) :])
    xqf = np.ascontiguousarray(
        xq[: 128 * (NTQ - 1)].reshape(max(NTQ - 1, 1), -1, W)[:, :128].transpose(1, 0, 2)
        if NTQ > 1
        else np.zeros((128, 1, W), F16)
    )
    xqr = np.ascontiguousarray(xq[128 * (NTQ - 1) :])

    mlin = np.zeros((NT * 128, 128), np.float32)
    mlin[np.arange(len(dsid)), dsid] = 1.0
    qbase = NTD * 128
    mlin[qbase + np.arange(len(qsid)), qsid] = 1.0
    onehot = np.ascontiguousarray(
        mlin.reshape(NT, 128, 128).transpose(1, 0, 2)
    ).astype(F16)
    return xdf, xdr, xqf, xqr, onehot


def kernel(hidden_states, W_doc, b_doc, W_query, b_query, query_len, seq_lens):
    hs = np.asarray(hidden_states, dtype=np.float32)
    hs2 = hs.reshape(B * L, H)
    wd = np.ascontiguousarray(
        np.broadcast_to(
            np.asarray(W_doc, np.float32).reshape(1, H).astype(F16), (128, H)
        )
    )
    wq = np.ascontiguousarray(
        np.broadcast_to(
            np.asarray(W_query, np.float32).reshape(1, H).astype(F16), (128, H)
        )
    )

    ex_map, plan, geom = _prepare(query_len, seq_lens)

    nc = _compiled.get(geom)
    if nc is None:
        nc = _build(*geom)
        _compiled[geom] = nc

    in_maps = []
    for c in range(NCORES):
        xdf, xdr, xqf, xqr, onehot = _stage_core(hs2, plan[c], geom)
        in_maps.append(
            {
                "xdf": xdf,
                "xdr": xdr,
                "xqf": xqf,
                "xqr": xqr,
                "onehot": onehot,
                "wbd": wd,
                "wbq": wq,
            }
        )

    from concourse.bass_utils import run_bass_kernel_spmd

    res = run_bass_kernel_spmd(nc, in_maps, list(range(NCORES)))

    doc = np.empty((B, D, H), np.float32)
    qp = np.empty((B, H), np.float32)
    for c in range(NCORES):
        r = res.results[c]
        dall = np.asarray(r["doc_out"], np.float32).reshape(EPB, D, H)
        qall = np.asarray(r["q_out"], np.float32)
        for k, e in enumerate(ex_map[c]):
            doc[e] = dall[k]
            qp[e] = qall[k]
    q_bcast = np.broadcast_to(qp[:, None, :], (B, D, H))
    return doc, q_bcast


# revision 8
# speedup vs baseline: 2.3200x; 1.7432x over previous
"""Trainium2 Bass kernel for nn_BertEncoder_403726926494.

Reference computation (per batch element):
  - ragged sentence extraction from hidden_states, masked-softmax attention
    pooling per sentence with W_doc            -> doc_pooled [B, D, H]
  - query extraction (rows 1..32), masked-softmax pooling with W_query
    broadcast over D                           -> q_bcast   [B, D, H]

Device strategy (SPMD, one program on 8 cores, 8 batch elements per core):
  - Dense token packing: each core receives ONE fp16 token stream = the
    concatenation of its 8 examples' doc-sentence tokens, padded to a 128
    boundary, followed by a copy of their query tokens.  Token t lands on
    SBUF partition t%128 of chunk t//128.  A trailing ones-column (col 768)
    rides along for the softmax denominators.
  - Per chunk: ONE fused DVE scalar_tensor_tensor computes xw = x*W and
    accumulates the per-token score s_t (fp32) in the same pass; ONE ACT
    Exp over a host-built log-mask (0 where token t belongs to pooling
    column m, -4096 elsewhere) with bias=s_t yields
    at[t,m] = exp(s_t)*onehot[t,m].
  - Doc pooling columns: 8 examples x 16 sentences = exactly 128 PE columns,
    so ONE PSUM accumulation group [128, 769] collects num|den for every
    sentence of the core across all doc chunks (fp16 matmuls, fp32 PSUM).
    Query chunks accumulate into a second small group [32, 769] (8 cols used).
  - out = num * (1/(den+eps)); results leave as fp16 and are scattered back
    to [B, D, H] on the host.  b_doc / b_query shift every score in a softmax
    segment equally, so they cancel and are ignored.
"""

import numpy as np
import ml_dtypes

B, L, H = 64, 512, 768
D, S, Q = 16, 64, 32
NCORES = 8
EPB = 8  # examples per core
NEG = -4096.0  # exp(NEG + s) == 0
DEN_EPS = 1.0e-30

F16 = np.float16
MASK_F8 = True  # log-mask as fp8e5 (halves mask DMA); flip to False if flaky
F8 = ml_dtypes.float8_e5m2 if MASK_F8 else np.float16

# chunks whose fused score op runs on GpSimd instead of DVE (tuning knob)
STT_GPS_CHUNKS: set = set()

_compiled: dict = {}


def _build(NTD, NTQ, REM_D, REM_Q):
    """Build + compile the SPMD Bass program for the given chunk geometry."""
    from contextlib import ExitStack

    import concourse.bacc as bacc
    import concourse.tile as tile
    from concourse import mybir

    f32 = mybir.dt.float32
    f16 = mybir.dt.float16
    f8 = mybir.dt.float8e5 if MASK_F8 else mybir.dt.float16
    MULT = mybir.AluOpType.mult
    ADD = mybir.AluOpType.add
    EXP = mybir.ActivationFunctionType.Exp
    COPY = mybir.ActivationFunctionType.Copy

    NT = NTD + NTQ
    W = H + 1  # 769: H data cols + ones col

    nc = bacc.Bacc(
        "TRN2", target_bir_lowering=False, debug=False, num_devices=NCORES
    )
    xdf = nc.dram_tensor("xdf", [128, max(NTD - 1, 1), W], f16, kind="ExternalInput").ap()
    xdr = nc.dram_tensor("xdr", [REM_D, W], f16, kind="ExternalInput").ap()
    xqf = nc.dram_tensor("xqf", [128, max(NTQ - 1, 1), W], f16, kind="ExternalInput").ap()
    xqr = nc.dram_tensor("xqr", [REM_Q, W], f16, kind="ExternalInput").ap()
    mask8 = nc.dram_tensor("mask8", [128, NT, 128], f8, kind="ExternalInput").ap()
    wbd = nc.dram_tensor("wbd", [128, H], f16, kind="ExternalInput").ap()
    wbq = nc.dram_tensor("wbq", [128, H], f16, kind="ExternalInput").ap()
    doc_out = nc.dram_tensor("doc_out", [128, H], f16, kind="ExternalOutput").ap()
    q_out = nc.dram_tensor("q_out", [EPB, H], f16, kind="ExternalOutput").ap()

    with tile.TileContext(nc) as tc, ExitStack() as ctx:
        const = ctx.enter_context(tc.tile_pool(name="const", bufs=1))
        atp = ctx.enter_context(tc.tile_pool(name="atp", bufs=3))
        scrp = ctx.enter_context(tc.tile_pool(name="scr", bufs=2))
        outp = ctx.enter_context(tc.tile_pool(name="outp", bufs=2))
        smallp = ctx.enter_context(tc.tile_pool(name="smallp", bufs=4))
        nump = ctx.enter_context(tc.tile_pool(name="nump", bufs=1, space="PSUM"))
        qnump = ctx.enter_context(tc.tile_pool(name="qnump", bufs=1, space="PSUM"))

        x = const.tile([128, NT, W], f16)
        mask_t = const.tile([128, NT, 128], f8)
        wb_d = const.tile([128, H], f16)
        wb_q = const.tile([128, H], f16)
        scol = const.tile([128, NT], f32)

        # zero the partial chunks up front so padding rows stay finite
        # (mask NEG keeps them out of the pools; engine partition access
        # must start at 0, so clear whole chunks, then partial-DMA)
        if REM_D < 128:
            nc.vector.memset(x[:, NTD - 1, :], 0.0)
        if REM_Q < 128:
            nc.vector.memset(x[:, NT - 1, :], 0.0)

        # ---- input DMAs; x + weights on the sync ring (stream order),
        # mask + outputs on the scalar ring so the two issue in parallel
        nc.scalar.dma_start(out=mask_t[:], in_=mask8[:])
        nc.sync.dma_start(out=wb_d[:], in_=wbd[:])
        nfd = NTD - 1
        bounds = sorted({min(g, nfd) for g in (4, 7, nfd)} | {0})
        for a, b in zip(bounds[:-1], bounds[1:]):
            if b > a:
                nc.sync.dma_start(out=x[:, a:b, :], in_=xdf[:, a:b, :])
        nc.sync.dma_start(out=x[0:REM_D, NTD - 1, :], in_=xdr[:])
        nc.sync.dma_start(out=wb_q[:], in_=wbq[:])
        if NTQ > 1:
            nc.sync.dma_start(out=x[:, NTD : NT - 1, :], in_=xqf[:])
        nc.sync.dma_start(out=x[0:REM_Q, NT - 1, :], in_=xqr[:])

        numg = nump.tile([128, 1024], f32, tag="num", name="num")
        qnumg = qnump.tile([32, 1024], f32, tag="qnum", name="qnum")

        def emit_chunk(c):
            wb = wb_d if c < NTD else wb_q
            mcols = 128 if c < NTD else 32
            psum = numg if c < NTD else qnumg
            start = c == 0 or c == NTD
            stop = c == NTD - 1 or c == NT - 1

            xw = scrp.tile([128, H], f16, tag="xw", name=f"xw{c}")
            eng = nc.gpsimd if c in STT_GPS_CHUNKS else nc.vector
            eng.scalar_tensor_tensor(
                out=xw[:], in0=x[:, c, 0:H], scalar=1.0, in1=wb[:],
                op0=MULT, op1=MULT, accum_out=scol[:, c : c + 1],
            )
            at = atp.tile([128, 128], f16, tag="at", name=f"at{c}")
            nc.scalar.activation(
                at[:, 0:mcols], mask_t[:, c, 0:mcols], EXP,
                bias=scol[:, c : c + 1], scale=1.0,
            )
            nc.tensor.matmul(
                psum[0:mcols, 0:512], at[:, 0:mcols], x[:, c, 0:512],
                start=start, stop=stop,
            )
            nc.tensor.matmul(
                psum[0:mcols, 512:W], at[:, 0:mcols], x[:, c, 512:W],
                start=start, stop=stop,
            )

        for c in range(NT):
            emit_chunk(c)

        # ---- doc finish: out = num / (den + eps) ----
        de = smallp.tile([128, 1], f32, tag="de", name="de")
        nc.vector.tensor_scalar(
            out=de[:], in0=numg[:, H : H + 1], scalar1=DEN_EPS, scalar2=None, op0=ADD
        )
        rec = smallp.tile([128, 1], f32, tag="rec", name="rec")
        nc.vector.reciprocal(rec[:], de[:])
        do = outp.tile([128, H], f16, tag="do", name="do")
        nc.scalar.activation(do[:], numg[:, 0:H], COPY, bias=0.0, scale=rec[:, 0:1])
        nc.scalar.dma_start(out=doc_out[:], in_=do[:])

        # ---- query finish ----
        qde = smallp.tile([EPB, 1], f32, tag="qde", name="qde")
        nc.vector.tensor_scalar(
            out=qde[:], in0=qnumg[0:EPB, H : H + 1], scalar1=DEN_EPS,
            scalar2=None, op0=ADD,
        )
        qrec = smallp.tile([EPB, 1], f32, tag="qrec", name="qrec")
        nc.vector.reciprocal(qrec[:], qde[:])
        qo = outp.tile([EPB, H], f16, tag="qo", name="qo")
        nc.scalar.activation(
            qo[:], qnumg[0:EPB, 0:H], COPY, bias=0.0, scale=qrec[:, 0:1]
        )
        nc.scalar.dma_start(out=q_out[:], in_=qo[:])

    nc.compile()
    return nc


def _prepare(query_len, seq_lens):
    """Host-side geometry: example->core assignment + per-core streams."""
    ql = np.asarray(query_len).astype(np.int64)
    sl = np.asarray(seq_lens).astype(np.int64)
    offs = ql[:, None] + 2 + np.cumsum(sl, axis=1) - sl  # [B, D] sentence starts
    doc_tok = sl.sum(axis=1)

    # greedy balance of total tokens into NCORES bins of EPB examples
    tot = doc_tok + ql
    order = np.argsort(-tot, kind="stable")
    ex_map = [[] for _ in range(NCORES)]
    loads = np.zeros(NCORES, np.int64)
    for e in order:
        cand = [c for c in range(NCORES) if len(ex_map[c]) < EPB]
        c = min(cand, key=lambda cc: (loads[cc], cc))
        ex_map[c].append(int(e))
        loads[c] += tot[e]

    drows = np.array([sum(doc_tok[e] for e in ex_map[c]) for c in range(NCORES)])
    qrows = np.array([sum(ql[e] for e in ex_map[c]) for c in range(NCORES)])
    NTD = int(max(-(-r // 128) for r in drows))
    NTQ = int(max(-(-r // 128) for r in qrows))
    REM_D = max(int(max(r - 128 * (NTD - 1) for r in drows)), 1)
    REM_Q = max(int(max(r - 128 * (NTQ - 1) for r in qrows)), 1)

    # per-core gather indices (into hs.reshape(B*L, H)) and pooling col ids
    plan = []
    for c in range(NCORES):
        didx, dsid, qidx, qsid = [], [], [], []
        for k, e in enumerate(ex_map[c]):
            for j in range(D):
                n = int(sl[e, j])
                if n == 0:
                    continue
                o = int(offs[e, j])
                didx.append(np.arange(e * L + o, e * L + o + n))
                dsid.append(np.full(n, 16 * k + j))
            n = int(ql[e])
            qidx.append(np.arange(e * L + 1, e * L + 1 + n))
            qsid.append(np.full(n, k))
        plan.append(
            (
                np.concatenate(didx),
                np.concatenate(dsid),
                np.concatenate(qidx),
                np.concatenate(qsid),
            )
        )
    return ex_map, plan, (NTD, NTQ, REM_D, REM_Q)


def _stage_core(hs2, plan_c, geom):
    NTD, NTQ, REM_D, REM_Q = geom
    NT = NTD + NTQ
    W = H + 1
    didx, dsid, qidx, qsid = plan_c
    nd_pad = 128 * (NTD - 1) + REM_D
    nq_pad = 128 * (NTQ - 1) + REM_Q

    def pack(idx, npad):
        xs = np.zeros((npad, W), F16)
        xs[: len(idx), 0:H] = hs2[idx]
        xs[: len(idx), H] = 1.0
        return xs

    xd = pack(didx, nd_pad)
    xq = pack(qidx, nq_pad)
    xdf = np.ascontiguousarray(
        xd[: 128 * (NTD - 1)].reshape(max(NTD - 1, 1), -1, W)[:, :128].transpose(1, 0, 2)
        if NTD > 1
        else np.zeros((128, 1, W), F16)
    )
    xdr = np.ascontiguousarray(xd[128 * (NTD - 1) :])
    xqf = np.ascontiguousarray(
        xq[: 128 * (NTQ - 1)].reshape(max(NTQ - 1, 1), -1, W)[:, :128].transpose(1, 0, 2)
        if NTQ > 1
        else np.zeros((128, 1, W), F16)
    )
    xqr = np.ascontiguousarray(xq[128 * (NTQ - 1) :])

    mlin = np.full((NT * 128, 128), NEG, np.float32)
    mlin[np.arange(len(dsid)), dsid] = 0.0
    qbase = NTD * 128
    mlin[qbase + np.arange(len(qsid)), qsid] = 0.0
    mask8 = np.ascontiguousarray(
        mlin.reshape(NT, 128, 128).transpose(1, 0, 2)
    ).astype(F8)
    return xdf, xdr, xqf, xqr, mask8


def kernel(hidden_states, W_doc, b_doc, W_query, b_query, query_len, seq_lens):
    hs = np.asarray(hidden_states, dtype=np.float32)
    hs2 = hs.reshape(B * L, H)
    wd = np.ascontiguousarray(
        np.broadcast_to(
            np.asarray(W_doc, np.float32).reshape(1, H).astype(F16), (128, H)
        )
    )
    wq = np.ascontiguousarray(
        np.broadcast_to(
            np.asarray(W_query, np.float32).reshape(1, H).astype(F16), (128, H)
        )
    )

    ex_map, plan, geom = _prepare(query_len, seq_lens)

    nc = _compiled.get(geom)
    if nc is None:
        nc = _build(*geom)
        _compiled[geom] = nc

    in_maps = []
    for c in range(NCORES):
        xdf, xdr, xqf, xqr, mask8 = _stage_core(hs2, plan[c], geom)
        in_maps.append(
            {
                "xdf": xdf,
                "xdr": xdr,
                "xqf": xqf,
                "xqr": xqr,
                "mask8": mask8,
                "wbd": wd,
                "wbq": wq,
            }
        )

    from concourse.bass_utils import run_bass_kernel_spmd

    res = run_bass_kernel_spmd(nc, in_maps, list(range(NCORES)))

    doc = np.empty((B, D, H), np.float32)
    qp = np.empty((B, H), np.float32)
    for c in range(NCORES):
        r = res.results[c]
        dall = np.asarray(r["doc_out"], np.float32).reshape(EPB, D, H)
        qall = np.asarray(r["q_out"], np.float32)
        for k, e in enumerate(ex_map[c]):
            doc[e] = dall[k]
            qp[e] = qall[k]
    q_bcast = np.broadcast_to(qp[:, None, :], (B, D, H))
    return doc, q_bcast
